# revision 8
# baseline (speedup 1.0000x reference)
"""Trainium2 Bass kernel for nn_ALNNLayer (ALNN attention-like layer).

Reference computation (per batch b, ref-time k, step l, feature d):
    dist  = |T[b,l,d] - r_k|                      r_k = linspace(0,48,13)
    kern  = exp(-relu(alpha_k) * dist)
    inten = relu(X * kern) = relu(X) * kern       (kern > 0)
    pre   = wt0*X + wt1*DT + wt2*inten + wt3*M + 4*bt
    lat   = relu(pre)
    out[b,k,d] = relu( sum_l wv*lat + 200*bv[k,d] )

Strategy: data-parallel over batch (8 cores x 8 batches). Per core,
SBUF layout is [l partitions, (b, d) free]; weights are broadcast over
b with stride-0 access patterns. The L-reduction runs on the
TensorEngine as a ones-vector matmul accumulating f32 in PSUM.
"""

import sys

for _p in ("/opt/trn_rl_repo", "/root/.axon_site/_ro/trn_rl_repo"):
    if _p not in sys.path:
        sys.path.append(_p)

import numpy as np

import concourse.bass as bass
import concourse.bacc as bacc
import concourse.tile as tile
from concourse import mybir
from concourse.bass_utils import run_bass_kernel_spmd

# Problem constants (fixed by the module being implemented)
B, L, D, K = 64, 200, 64, 4 * 3 + 1  # K = 13
NCORES = 8
BLOC = B // NCORES  # 8
PRIOR_HOURS = 48.0
REF_TIME = np.linspace(0.0, PRIOR_HOURS, K).astype(np.float32)  # r_k

L0 = 128  # first l-tile partitions
L1 = L - L0  # 72

F32 = mybir.dt.float32
AX = mybir.AluOpType


def _bcast_b(ap, nb=BLOC):
    """[P, D] access pattern -> [P, nb, D] with a stride-0 b dim."""
    assert len(ap.ap) == 2, ap.ap
    return bass.AP(tensor=ap.tensor, offset=ap.offset, ap=[ap.ap[0], [0, nb], ap.ap[1]])


def build_bass():
    nc = bacc.Bacc("TRN2", target_bir_lowering=False, debug=False)

    T_d = nc.declare_dram_parameter("T", [BLOC, L, D], F32, isOutput=False)
    X_d = nc.declare_dram_parameter("X", [BLOC, L, D], F32, isOutput=False)
    DT_d = nc.declare_dram_parameter("DTm", [BLOC, L, D], F32, isOutput=False)
    M_d = nc.declare_dram_parameter("Mm", [BLOC, L, D], F32, isOutput=False)
    # Packed per-k weights: [K, L, 6, D] with f = (wt0, wt1, wt2, wt3, 4*bt, wv)
    W_d = nc.declare_dram_parameter("W", [K, L, 6, D], F32, isOutput=False)
    # SC: [128, 2K] = [-relu(alpha) | -r_k], replicated across partitions
    S_d = nc.declare_dram_parameter("S", [128, 2 * K], F32, isOutput=False)
    BV_d = nc.declare_dram_parameter("BV", [K, D], F32, isOutput=False)  # 200*b_v
    # Column-selector for the L-sum matmul: ESEL[:, k*K + m] = (m == k)
    ONE_d = nc.declare_dram_parameter("ONES", [128, K * K], F32, isOutput=False)
    out_d = nc.declare_dram_parameter("out", [BLOC, K, D], F32, isOutput=True)

    from contextlib import ExitStack

    with tile.TileContext(nc) as tc, ExitStack() as ctx:
        const = ctx.enter_context(tc.tile_pool(name="const", bufs=1))
        wpool = ctx.enter_context(tc.tile_pool(name="wpool", bufs=3))
        tmp = ctx.enter_context(tc.tile_pool(name="tmp", bufs=2))
        psum = ctx.enter_context(tc.tile_pool(name="psum", bufs=1, space="PSUM"))

        # ---- resident inputs, [l, b, d] ----
        def load_input(dram):
            t0 = const.tile([L0, BLOC, D], F32, tag=dram.name + "0")
            t1 = const.tile([L1, BLOC, D], F32, tag=dram.name + "1")
            src = dram[:].rearrange("b l d -> l b d")
            nc.sync.dma_start(out=t0[:], in_=src[0:L0])
            nc.sync.dma_start(out=t1[:], in_=src[L0:L])
            return t0, t1

        Tt = load_input(T_d)
        Xt = load_input(X_d)
        DTt = load_input(DT_d)
        Mt = load_input(M_d)

        S_sb = const.tile([128, 2 * K], F32)
        nc.sync.dma_start(out=S_sb[:], in_=S_d[:])
        BV_sb = const.tile([K, D], F32)
        nc.sync.dma_start(out=BV_sb[:], in_=BV_d[:])
        ones_sb = const.tile([128, K * K], F32)
        nc.sync.dma_start(out=ones_sb[:], in_=ONE_d[:])

        # relu(X), computed once
        RXt = []
        for i, P in enumerate((L0, L1)):
            rx = const.tile([P, BLOC, D], F32, tag=f"RX{i}")
            nc.vector.tensor_scalar_max(rx[:], Xt[i][:], 0.0)
            RXt.append(rx)

        # ---- per-k pipeline ----
        # PE matmul PSUM outputs must start at partition 0/32/64, so the
        # per-k L-sum uses a [P, K] selector lhsT whose column k is ones:
        # row k of po gets the sum, all other rows accumulate zeros.
        osb = const.tile([K, BLOC, D], F32)
        po = psum.tile([K, BLOC, D], F32)

        for k in range(K):
            wk = []
            for i, (P, lo) in enumerate(((L0, 0), (L1, L0))):
                w = wpool.tile([P, 6, D], F32, tag=f"wk{i}")
                nc.sync.dma_start(out=w[:], in_=W_d[k, lo : lo + P])
                wk.append(w)

            for i, P in enumerate((L0, L1)):
                w = wk[i]
                wt0 = _bcast_b(w[:, 0, :])
                wt1 = _bcast_b(w[:, 1, :])
                wt2 = _bcast_b(w[:, 2, :])
                wt3 = _bcast_b(w[:, 3, :])
                bt4 = _bcast_b(w[:, 4, :])
                wv = _bcast_b(w[:, 5, :])

                dist = tmp.tile([P, BLOC, D], F32, tag=f"dist{i}")
                nc.scalar.activation(
                    dist[:], Tt[i][:], mybir.ActivationFunctionType.Abs,
                    bias=S_sb[:P, K + k : K + k + 1], scale=1.0,
                )
                kern = tmp.tile([P, BLOC, D], F32, tag=f"kern{i}")
                nc.scalar.activation(
                    kern[:], dist[:], mybir.ActivationFunctionType.Exp,
                    scale=S_sb[:P, k : k + 1],
                )
                # Q = wt2 * relu(X) * kern
                pq = tmp.tile([P, BLOC, D], F32, tag=f"pq{i}")
                nc.vector.tensor_tensor(pq[:], RXt[i][:], kern[:], AX.mult)
                qq = tmp.tile([P, BLOC, D], F32, tag=f"qq{i}")
                nc.vector.tensor_tensor(qq[:], pq[:], wt2, AX.mult)
                # affine terms
                s0 = tmp.tile([P, BLOC, D], F32, tag=f"s0{i}")
                nc.vector.tensor_tensor(s0[:], Xt[i][:], wt0, AX.mult)
                s1 = tmp.tile([P, BLOC, D], F32, tag=f"s1{i}")
                nc.vector.tensor_tensor(s1[:], DTt[i][:], wt1, AX.mult)
                s2 = tmp.tile([P, BLOC, D], F32, tag=f"s2{i}")
                nc.vector.tensor_tensor(s2[:], Mt[i][:], wt3, AX.mult)
                u = tmp.tile([P, BLOC, D], F32, tag=f"u{i}")
                nc.vector.tensor_tensor(u[:], s0[:], s1[:], AX.add)
                v = tmp.tile([P, BLOC, D], F32, tag=f"v{i}")
                nc.vector.tensor_tensor(v[:], s2[:], qq[:], AX.add)
                w2 = tmp.tile([P, BLOC, D], F32, tag=f"w2{i}")
                nc.vector.tensor_tensor(w2[:], u[:], v[:], AX.add)
                pre = tmp.tile([P, BLOC, D], F32, tag=f"pre{i}")
                nc.vector.tensor_tensor(pre[:], w2[:], bt4, AX.add)
                # z = relu(pre) * wv in one fused op
                z = tmp.tile([P, BLOC, D], F32, tag=f"z{i}")
                nc.vector.scalar_tensor_tensor(
                    z[:], pre[:], 0.0, wv, op0=AX.max, op1=AX.mult
                )
                # sum over l into psum row k via the selector matmul
                nc.tensor.matmul(
                    po[:, :, :],
                    ones_sb[:P, k * K : (k + 1) * K],
                    z[:],
                    start=(k == 0 and i == 0),
                    stop=(k == K - 1 and i == 1),
                )

        # ---- epilogue: out = relu(po + 200*bv) ----
        bvb = _bcast_b(BV_sb[:])
        nc.vector.tensor_tensor(osb[:], po[:], bvb, AX.add)
        nc.vector.tensor_scalar_max(osb[:], osb[:], 0.0)
        nc.sync.dma_start(out=out_d[:].rearrange("b k d -> k b d"), in_=osb[:])

    nc.compile()
    return nc


_NC_CACHE = None


def _get_nc():
    global _NC_CACHE
    if _NC_CACHE is None:
        _NC_CACHE = build_bass()
    return _NC_CACHE


def make_in_maps(X, T, M, DT, alpha, w_v, w_t, b_v, b_t):
    X = np.asarray(X, np.float32)
    T = np.asarray(T, np.float32)
    M = np.asarray(M, np.float32)
    DT = np.asarray(DT, np.float32)
    w_t = np.asarray(w_t, np.float32)
    w_v = np.asarray(w_v, np.float32)
    b_t = np.asarray(b_t, np.float32)
    b_v = np.asarray(b_v, np.float32)
    alpha = np.asarray(alpha, np.float32)

    W = np.empty((K, L, 6, D), np.float32)
    W[:, :, 0] = w_t[:, :, :, 0]
    W[:, :, 1] = w_t[:, :, :, 1]
    W[:, :, 2] = w_t[:, :, :, 2]
    W[:, :, 3] = w_t[:, :, :, 3]
    W[:, :, 4] = 4.0 * b_t[:, :, :, 0]
    W[:, :, 5] = w_v
    S = np.tile(
        np.concatenate(
            [-np.maximum(alpha.reshape(1, K), 0.0), -REF_TIME.reshape(1, K)], axis=1
        ),
        (128, 1),
    ).astype(np.float32)
    BV = (float(L) * b_v[:, 0, :]).astype(np.float32)
    ONES = np.zeros((128, K, K), np.float32)
    for k in range(K):
        ONES[:, k, k] = 1.0
    ONES = ONES.reshape(128, K * K)

    in_maps = []
    for c in range(NCORES):
        b0 = c * BLOC
        in_maps.append(
            {
                "T": np.ascontiguousarray(T[b0 : b0 + BLOC]),
                "X": np.ascontiguousarray(X[b0 : b0 + BLOC]),
                "DTm": np.ascontiguousarray(DT[b0 : b0 + BLOC]),
                "Mm": np.ascontiguousarray(M[b0 : b0 + BLOC]),
                "W": W,
                "S": S,
                "BV": BV,
                "ONES": ONES,
            }
        )
    return in_maps


def kernel(X, T, M, DT, alpha, w_v, w_t, b_v, b_t):
    nc = _get_nc()
    in_maps = make_in_maps(X, T, M, DT, alpha, w_v, w_t, b_v, b_t)
    res = run_bass_kernel_spmd(nc, in_maps, core_ids=list(range(NCORES)))
    out = np.concatenate([res.results[c]["out"] for c in range(NCORES)], axis=0)
    return out.astype(np.float32)


# revision 10
# speedup vs baseline: 1.7183x; 1.7183x over previous
"""Trainium2 Bass kernel for nn_ALNNLayer (ALNN attention-like layer).

Reference computation (per batch b, ref-time k, step l, feature d):
    dist  = |T[b,l,d] - r_k|                      r_k = linspace(0,48,13)
    kern  = exp(-relu(alpha_k) * dist)
    inten = relu(X * kern) = relu(X) * kern       (kern > 0)
    pre   = wt0*X + wt1*DT + wt2*inten + wt3*M + 4*bt
    lat   = relu(pre)
    out[b,k,d] = relu( sum_l wv*lat + 200*bv[k,d] )

Strategy: data-parallel over batch (8 cores x 8 batches). Per core the
SBUF layout is [100 l-partitions, (j=l//100, b, d) free] = [100, 1024];
weights broadcast over b with stride-0 access patterns. Work is spread
over all engines:
  - ScalarE (ACT): |T-r_k|, exp, and relu fused into the PSUM eviction
  - VectorE (DVE): bf16 products at 2x mode
  - GPSIMD: one bf16 product per k
  - TensorE (PE): term accumulation via identity matmuls into PSUM and
    the L-reduction via a k-column selector matmul (PSUM outputs must
    start at partition 0, so column k of the selector carries the ones
    and the other 12 output rows just accumulate zeros).
k's with relu(alpha_k) == 0 skip dist/exp entirely (kern == 1); the
NEFF is compiled per alpha-sign-pattern, so this stays input-correct.
"""

import sys

for _p in ("/opt/trn_rl_repo", "/root/.axon_site/_ro/trn_rl_repo"):
    if _p not in sys.path:
        sys.path.append(_p)

import numpy as np
import ml_dtypes

import concourse.bass as bass
import concourse.bacc as bacc
import concourse.tile as tile
from concourse import mybir
from concourse.bass_utils import run_bass_kernel_spmd

# Problem constants (fixed by the module being implemented)
B, L, D, K = 64, 200, 64, 13
NCORES = 8
BLOC = B // NCORES  # 8
PRIOR_HOURS = 48.0
REF_TIME = np.linspace(0.0, PRIOR_HOURS, K).astype(np.float32)  # r_k

LP = 100            # l partitions
LJ = 2              # l super-tiles (l = j*LP + p)
FD = LJ * BLOC * D  # 1024 free elements per partition

F32 = mybir.dt.float32
BF16 = mybir.dt.bfloat16
AX = mybir.AluOpType
AF = mybir.ActivationFunctionType
NPBF = ml_dtypes.bfloat16

# ---- tuning knobs ----
GPS_OPS = ("s1",)      # which of s0/s1/s2 run on GPSIMD
DIST_ENGINE = "act"    # "dve" | "act" | "gps"
BT4_ON_PE = True       # accumulate 4*b_t via PE broadcast-AP matmul


def _bcast_b(ap, nb=BLOC):
    """Insert a stride-0 b dim before the last free dim of an AP."""
    return bass.AP(
        tensor=ap.tensor, offset=ap.offset,
        ap=list(ap.ap[:-1]) + [[0, nb], ap.ap[-1]],
    )


def build_bass(nonzero):
    """nonzero: tuple of bool per k — whether relu(alpha_k) > 0."""
    nc = bacc.Bacc("TRN2", target_bir_lowering=False, debug=False)

    T_d = nc.declare_dram_parameter("T", [BLOC, L, D], F32, isOutput=False)
    X_d = nc.declare_dram_parameter("X", [BLOC, L, D], BF16, isOutput=False)
    DT_d = nc.declare_dram_parameter("DTm", [BLOC, L, D], BF16, isOutput=False)
    M_d = nc.declare_dram_parameter("Mm", [BLOC, L, D], BF16, isOutput=False)
    # Packed per-k weights: [K, LP, LJ, 6, D], f = (wt0, wt1, wt2, wt3, 4bt, wv)
    W_d = nc.declare_dram_parameter("W", [K, LP, LJ, 6, D], BF16, isOutput=False)
    # SC: [128, 2K] f32 = [-relu(alpha) | -r_k], replicated across partitions
    S_d = nc.declare_dram_parameter("S", [128, 2 * K], F32, isOutput=False)
    BV_d = nc.declare_dram_parameter("BV", [K, D], F32, isOutput=False)  # 200*b_v
    # ESEL: [128, K*K + 128] bf16 = per-k ones-selector columns | identity
    E_d = nc.declare_dram_parameter("ESEL", [128, K * K + 128], BF16, isOutput=False)
    out_d = nc.declare_dram_parameter("out", [BLOC, K, D], F32, isOutput=True)

    from contextlib import ExitStack

    with tile.TileContext(nc) as tc, ExitStack() as ctx:
        const = ctx.enter_context(tc.tile_pool(name="const", bufs=1))
        wpool = ctx.enter_context(tc.tile_pool(name="wpool", bufs=3))
        tmp = ctx.enter_context(tc.tile_pool(name="tmp", bufs=3))
        psum = ctx.enter_context(tc.tile_pool(name="psum", bufs=2, space="PSUM"))
        psum1 = ctx.enter_context(tc.tile_pool(name="psum1", bufs=1, space="PSUM"))

        # ---- resident inputs as [LP, (j, b, d)] ----
        def load_input(dram, dt):
            t = const.tile([LP, LJ, BLOC, D], dt, tag=dram.name)
            src = dram[:].rearrange("b (j p) d -> j p b d", j=LJ)
            for j in range(LJ):
                nc.sync.dma_start(out=t[:, j], in_=src[j])
            return t

        Tt = load_input(T_d, F32)
        Xt = load_input(X_d, BF16)
        DTt = load_input(DT_d, BF16)
        Mt = load_input(M_d, BF16)

        S_sb = const.tile([128, 2 * K], F32)
        nc.sync.dma_start(out=S_sb[:], in_=S_d[:])
        BV_sb = const.tile([K, D], F32)
        nc.sync.dma_start(out=BV_sb[:], in_=BV_d[:])
        E_sb = const.tile([128, K * K + 128], BF16)
        nc.sync.dma_start(out=E_sb[:], in_=E_d[:])
        eye = E_sb[:LP, K * K : K * K + LP]

        RXt = const.tile([LP, LJ, BLOC, D], BF16, tag="RX")
        nc.vector.tensor_scalar_max(RXt[:], Xt[:], 0.0)

        osb = const.tile([K, BLOC, D], F32)
        po = psum1.tile([K, BLOC, D], F32)  # L-sums, one bank, rows = k

        for k in range(K):
            w = wpool.tile([LP, LJ, 6, D], BF16, tag="wk")
            nc.sync.dma_start(out=w[:], in_=W_d[k])
            wt = [_bcast_b(w[:, :, f, :]) for f in range(6)]

            # ---- kern term: Q = wt2 * relu(X) * kern ----
            if nonzero[k]:
                dist = tmp.tile([LP, LJ, BLOC, D], F32, tag="dist")
                if DIST_ENGINE == "act":
                    nc.scalar.activation(
                        dist[:], Tt[:], AF.Abs,
                        bias=S_sb[:LP, K + k : K + k + 1], scale=1.0,
                    )
                else:
                    eng = nc.vector if DIST_ENGINE == "dve" else nc.gpsimd
                    eng.tensor_scalar(
                        dist[:], Tt[:], float(REF_TIME[k]), 0.0,
                        op0=AX.subtract, op1=AX.abs_max,
                    )
                kern = tmp.tile([LP, LJ, BLOC, D], BF16, tag="kern")
                nc.scalar.activation(
                    kern[:], dist[:], AF.Exp, scale=S_sb[:LP, k : k + 1]
                )
                kw = tmp.tile([LP, LJ, BLOC, D], BF16, tag="kw")
                nc.vector.tensor_tensor(kw[:], kern[:], wt[2], AX.mult)
                qin = kw[:]
            else:
                qin = wt[2]
            Q = tmp.tile([LP, LJ, BLOC, D], BF16, tag="Q")
            nc.vector.tensor_tensor(Q[:], RXt[:], qin, AX.mult)

            # ---- affine products ----
            terms = [Q]
            for nm, dat, wi in (("s0", Xt, 0), ("s1", DTt, 1), ("s2", Mt, 3)):
                s = tmp.tile([LP, LJ, BLOC, D], BF16, tag=nm)
                eng = nc.gpsimd if nm in GPS_OPS else nc.vector
                eng.tensor_tensor(s[:], dat[:], wt[wi], AX.mult)
                terms.append(s)

            # ---- pre = sum(terms) + 4bt via PE identity matmuls ----
            pre = psum.tile([LP, LJ, BLOC, D], F32, tag="pre")
            rhss = [t[:, j] for t in terms for j in range(LJ)]
            if BT4_ON_PE:
                rhss += [wt[4][:, j] for j in range(LJ)]
            else:
                bt4m = tmp.tile([LP, LJ, BLOC, D], BF16, tag="bt4m")
                nc.vector.tensor_copy(bt4m[:], wt[4])
                rhss += [bt4m[:, j] for j in range(LJ)]
            nterm = len(rhss) // LJ
            for ti in range(nterm):
                for j in range(LJ):
                    nc.tensor.matmul(
                        pre[:, j],
                        eye,
                        rhss[ti * LJ + j],
                        start=(ti == 0),
                        stop=(ti == nterm - 1),
                    )

            # ---- lat = relu(pre) on the PSUM eviction, then z = lat*wv ----
            lat = tmp.tile([LP, LJ, BLOC, D], BF16, tag="lat")
            nc.scalar.activation(lat[:], pre[:], AF.Relu)
            z = tmp.tile([LP, LJ, BLOC, D], BF16, tag="z")
            nc.vector.tensor_tensor(z[:], lat[:], wt[5], AX.mult)

            # ---- sum over l into po row k: selector matmul ----
            for j in range(LJ):
                nc.tensor.matmul(
                    po[:, :, :],
                    E_sb[:LP, k * K : (k + 1) * K],
                    z[:, j],
                    start=(k == 0 and j == 0),
                    stop=(k == K - 1 and j == LJ - 1),
                )

        # ---- epilogue: out = relu(po + 200*bv) ----
        bvb = _bcast_b(BV_sb[:])
        nc.vector.tensor_tensor(osb[:], po[:], bvb, AX.add)
        nc.vector.tensor_scalar_max(osb[:], osb[:], 0.0)
        nc.sync.dma_start(out=out_d[:].rearrange("b k d -> k b d"), in_=osb[:])

    nc.compile()
    return nc


_NC_CACHE = {}


def _get_nc(nonzero):
    key = tuple(nonzero)
    if key not in _NC_CACHE:
        _NC_CACHE[key] = build_bass(key)
    return _NC_CACHE[key]


def make_in_maps(X, T, M, DT, alpha, w_v, w_t, b_v, b_t):
    X = np.asarray(X, np.float32)
    T = np.asarray(T, np.float32)
    M = np.asarray(M, np.float32)
    DT = np.asarray(DT, np.float32)
    w_t = np.asarray(w_t, np.float32)
    w_v = np.asarray(w_v, np.float32)
    b_t = np.asarray(b_t, np.float32)
    b_v = np.asarray(b_v, np.float32)
    alpha = np.asarray(alpha, np.float32).reshape(K)

    W = np.empty((K, L, 6, D), np.float32)
    W[:, :, 0] = w_t[:, :, :, 0]
    W[:, :, 1] = w_t[:, :, :, 1]
    W[:, :, 2] = w_t[:, :, :, 2]
    W[:, :, 3] = w_t[:, :, :, 3]
    W[:, :, 4] = 4.0 * b_t[:, :, :, 0]
    W[:, :, 5] = w_v
    # [K, L, 6, D] -> [K, LP, LJ, 6, D] with l = j*LP + p
    W = W.reshape(K, LJ, LP, 6, D).transpose(0, 2, 1, 3, 4)
    W = np.ascontiguousarray(W).astype(NPBF)

    S = np.tile(
        np.concatenate(
            [-np.maximum(alpha.reshape(1, K), 0.0), -REF_TIME.reshape(1, K)], axis=1
        ),
        (128, 1),
    ).astype(np.float32)
    BV = (float(L) * b_v[:, 0, :]).astype(np.float32)
    ESEL = np.zeros((128, K * K + 128), np.float32)
    for k in range(K):
        ESEL[:, k * K + k] = 1.0
    ESEL[:, K * K :] = np.eye(128, dtype=np.float32)
    ESEL = ESEL.astype(NPBF)

    in_maps = []
    for c in range(NCORES):
        b0 = c * BLOC
        in_maps.append(
            {
                "T": np.ascontiguousarray(T[b0 : b0 + BLOC]),
                "X": np.ascontiguousarray(X[b0 : b0 + BLOC]).astype(NPBF),
                "DTm": np.ascontiguousarray(DT[b0 : b0 + BLOC]).astype(NPBF),
                "Mm": np.ascontiguousarray(M[b0 : b0 + BLOC]).astype(NPBF),
                "W": W,
                "S": S,
                "BV": BV,
                "ESEL": ESEL,
            }
        )
    return in_maps, tuple(bool(a > 0) for a in alpha)


def kernel(X, T, M, DT, alpha, w_v, w_t, b_v, b_t):
    in_maps, nonzero = make_in_maps(X, T, M, DT, alpha, w_v, w_t, b_v, b_t)
    nc = _get_nc(nonzero)
    res = run_bass_kernel_spmd(nc, in_maps, core_ids=list(range(NCORES)))
    out = np.concatenate([res.results[c]["out"] for c in range(NCORES)], axis=0)
    return out.astype(np.float32)


# revision 11
# speedup vs baseline: 1.9381x; 1.1279x over previous
"""Trainium2 Bass kernel for nn_ALNNLayer (ALNN attention-like layer).

Reference computation (per batch b, ref-time k, step l, feature d):
    dist  = |T[b,l,d] - r_k|                      r_k = linspace(0,48,13)
    kern  = exp(-relu(alpha_k) * dist)
    inten = relu(X * kern) = relu(X) * kern       (kern > 0)
    pre   = wt0*X + wt1*DT + wt2*inten + wt3*M + 4*bt
    lat   = relu(pre)
    out[b,k,d] = relu( sum_l wv*lat + 200*bv[k,d] )

Strategy: data-parallel over batch (8 cores x 8 batches). Per core the
SBUF layout is [100 l-partitions, (j=l//100, b, d) free]; weights are
broadcast over b with stride-0 access patterns. Engine split:
  - VectorE: one packed bf16 multiply computes all four products
    (X*wt0 | DT*wt1 | M*wt3 | relu(X)*wt2) in a single [100, 4096] op,
    plus kern-apply (nonzero alpha_k only) and the wv multiply
  - ScalarE: |T-r_k|, exp, and relu fused into the PSUM eviction
  - TensorE: term summation via identity matmuls accumulating in PSUM,
    and the L-reduction via a k-column selector matmul (PSUM outputs
    must start at partition 0, so column k of the selector carries the
    ones and the other 12 output rows accumulate zeros)
k's with relu(alpha_k) == 0 skip dist/exp/kern entirely (kern == 1);
the NEFF is compiled per alpha-sign-pattern, so this stays correct for
any inputs.
"""

import sys

for _p in ("/opt/trn_rl_repo", "/root/.axon_site/_ro/trn_rl_repo"):
    if _p not in sys.path:
        sys.path.append(_p)

import numpy as np
import ml_dtypes

import concourse.bass as bass
import concourse.bacc as bacc
import concourse.tile as tile
from concourse import mybir
from concourse.bass_utils import run_bass_kernel_spmd

B, L, D, K = 64, 200, 64, 13
NCORES = 8
BLOC = B // NCORES  # 8
PRIOR_HOURS = 48.0
REF_TIME = np.linspace(0.0, PRIOR_HOURS, K).astype(np.float32)

LP = 100            # l partitions
LJ = 2              # l super-tiles (l = j*LP + p)
FD = LJ * BLOC * D  # 1024 free elements per partition per (k, f)
NF = 4              # packed product features: X, DT, M, relu(X)

F32 = mybir.dt.float32
BF16 = mybir.dt.bfloat16
AX = mybir.AluOpType
AF = mybir.ActivationFunctionType
NPBF = ml_dtypes.bfloat16

# ---- tuning knobs ----
DIST_ENGINE = "act"   # "dve" | "act" | "gps"
PAIR_ADDS = 0         # 0..2 pairwise DVE adds to offload PE matmuls
PACK_SPLIT = 1        # 1 = single packed product TT, 2 = two halves


def _bc(ap, nb=BLOC):
    """Insert a stride-0 b dim before the last free dim of an AP."""
    return bass.AP(
        tensor=ap.tensor, offset=ap.offset,
        ap=list(ap.ap[:-1]) + [[0, nb], ap.ap[-1]],
    )


def build_bass(nonzero):
    """nonzero: tuple of bool per k — whether relu(alpha_k) > 0."""
    nc = bacc.Bacc("TRN2", target_bir_lowering=False, debug=False)

    T_d = nc.declare_dram_parameter("T", [BLOC, L, D], F32, isOutput=False)
    X_d = nc.declare_dram_parameter("X", [BLOC, L, D], BF16, isOutput=False)
    DT_d = nc.declare_dram_parameter("DTm", [BLOC, L, D], BF16, isOutput=False)
    M_d = nc.declare_dram_parameter("Mm", [BLOC, L, D], BF16, isOutput=False)
    # per-k weights: [K, LP, 4, LJ, D] products (wt0, wt1, wt3, wt2)
    #              | [K, LP, 2, LJ, D] extras (4bt, wv)
    W_d = nc.declare_dram_parameter("W", [K, LP, NF + 2, LJ, D], BF16, isOutput=False)
    S_d = nc.declare_dram_parameter("S", [128, 2 * K], F32, isOutput=False)
    BV_d = nc.declare_dram_parameter("BV", [K, D], F32, isOutput=False)  # 200*b_v
    E_d = nc.declare_dram_parameter("ESEL", [128, K * K + 128], BF16, isOutput=False)
    out_d = nc.declare_dram_parameter("out", [BLOC, K, D], F32, isOutput=True)

    from contextlib import ExitStack

    with tile.TileContext(nc) as tc, ExitStack() as ctx:
        const = ctx.enter_context(tc.tile_pool(name="const", bufs=1))
        wpool = ctx.enter_context(tc.tile_pool(name="wpool", bufs=3))
        tmp = ctx.enter_context(tc.tile_pool(name="tmp", bufs=3))
        psum = ctx.enter_context(tc.tile_pool(name="psum", bufs=3, space="PSUM"))
        psum1 = ctx.enter_context(tc.tile_pool(name="psum1", bufs=1, space="PSUM"))

        # ---- resident data, packed [LP, (f, j, b, d)] ----
        Dp = const.tile([LP, NF, LJ, BLOC, D], BF16, tag="Dp")
        for f, dram in ((0, X_d), (1, DT_d), (2, M_d)):
            src = dram[:].rearrange("b (j p) d -> j p b d", j=LJ)
            for j in range(LJ):
                nc.sync.dma_start(out=Dp[:, f, j], in_=src[j])
        Tt = const.tile([LP, LJ, BLOC, D], F32, tag="T")
        srcT = T_d[:].rearrange("b (j p) d -> j p b d", j=LJ)
        for j in range(LJ):
            nc.sync.dma_start(out=Tt[:, j], in_=srcT[j])

        S_sb = const.tile([128, 2 * K], F32)
        nc.sync.dma_start(out=S_sb[:], in_=S_d[:])
        BV_sb = const.tile([K, D], F32)
        nc.sync.dma_start(out=BV_sb[:], in_=BV_d[:])
        E_sb = const.tile([128, K * K + 128], BF16)
        nc.sync.dma_start(out=E_sb[:], in_=E_d[:])
        eye = E_sb[:LP, K * K : K * K + LP]

        # f3 slot <- relu(X)
        nc.vector.tensor_scalar_max(Dp[:, NF - 1], Dp[:, 0], 0.0)

        osb = const.tile([K, BLOC, D], F32)
        po = psum1.tile([K, BLOC, D], F32)  # L-sums, one bank, rows = k

        for k in range(K):
            w = wpool.tile([LP, NF + 2, LJ, D], BF16, tag="wk")
            nc.sync.dma_start(out=w[:], in_=W_d[k])

            # ---- all four products in one packed multiply ----
            Sp = tmp.tile([LP, NF, LJ, BLOC, D], BF16, tag="Sp")
            nparts = PACK_SPLIT
            fstep = NF // nparts
            for half in range(nparts):
                f0 = half * fstep
                wap = bass.AP(
                    tensor=w[:].tensor,
                    offset=w[:].offset + f0 * LJ * D,
                    ap=[w[:].ap[0], [LJ * D, fstep], [D, LJ], [0, BLOC], [1, D]],
                )
                nc.vector.tensor_tensor(
                    Sp[:, f0 : f0 + fstep], Dp[:, f0 : f0 + fstep], wap, AX.mult
                )

            # ---- kern-apply for nonzero alpha_k ----
            if nonzero[k]:
                dist = tmp.tile([LP, LJ, BLOC, D], F32, tag="dist")
                if DIST_ENGINE == "act":
                    nc.scalar.activation(
                        dist[:], Tt[:], AF.Abs,
                        bias=S_sb[:LP, K + k : K + k + 1], scale=1.0,
                    )
                else:
                    eng = nc.vector if DIST_ENGINE == "dve" else nc.gpsimd
                    eng.tensor_scalar(
                        dist[:], Tt[:], float(REF_TIME[k]), 0.0,
                        op0=AX.subtract, op1=AX.abs_max,
                    )
                kern = tmp.tile([LP, LJ, BLOC, D], BF16, tag="kern")
                nc.scalar.activation(
                    kern[:], dist[:], AF.Exp, scale=S_sb[:LP, k : k + 1]
                )
                Q = tmp.tile([LP, LJ, BLOC, D], BF16, tag="Q")
                nc.vector.tensor_tensor(Q[:], Sp[:, NF - 1], kern[:], AX.mult)
                qterm = Q
            else:
                qterm = None  # Sp[:, NF-1] is already the full term

            # ---- pre = sum of terms (+ 4bt) via PE identity matmuls ----
            terms = [
                (Sp[:, 0, j] if f == 0 else
                 Sp[:, 1, j] if f == 1 else
                 Sp[:, 2, j] if f == 2 else
                 (qterm[:, j] if qterm is not None else Sp[:, NF - 1, j]) if f == 3
                 else _bc(w[:, NF, j]))
                for f in range(5)
                for j in range(LJ)
            ]
            # optional pairwise DVE adds to offload the PE
            np_pairs = PAIR_ADDS
            if np_pairs >= 1:
                u = tmp.tile([LP, LJ, BLOC, D], BF16, tag="u")
                nc.vector.tensor_tensor(u[:], Sp[:, 0], Sp[:, 1], AX.add)
                terms = [u[:, j] for j in range(LJ)] + terms[2 * LJ :]

            pre = psum.tile([LP, LJ, BLOC, D], F32, tag="pre")
            nt = len(terms) // LJ
            for ti in range(nt):
                for j in range(LJ):
                    nc.tensor.matmul(
                        pre[:, j], eye, terms[ti * LJ + j],
                        start=(ti == 0), stop=(ti == nt - 1),
                    )

            # ---- relu on the PSUM eviction, then z = lat * wv ----
            lat = tmp.tile([LP, LJ, BLOC, D], BF16, tag="lat")
            nc.scalar.activation(lat[:], pre[:], AF.Relu)
            z = tmp.tile([LP, LJ, BLOC, D], BF16, tag="z")
            nc.vector.tensor_tensor(z[:], lat[:], _bc(w[:, NF + 1]), AX.mult)

            # ---- sum over l into po row k: selector matmul ----
            for j in range(LJ):
                nc.tensor.matmul(
                    po[:, :, :],
                    E_sb[:LP, k * K : (k + 1) * K],
                    z[:, j],
                    start=(k == 0 and j == 0),
                    stop=(k == K - 1 and j == LJ - 1),
                )

        # ---- epilogue: out = relu(po + 200*bv) ----
        nc.vector.tensor_tensor(osb[:], po[:], _bc(BV_sb[:]), AX.add)
        nc.vector.tensor_scalar_max(osb[:], osb[:], 0.0)
        nc.sync.dma_start(out=out_d[:].rearrange("b k d -> k b d"), in_=osb[:])

    nc.compile()
    return nc


_NC_CACHE = {}


def _get_nc(nonzero):
    key = tuple(nonzero)
    if key not in _NC_CACHE:
        _NC_CACHE[key] = build_bass(key)
    return _NC_CACHE[key]


def make_in_maps(X, T, M, DT, alpha, w_v, w_t, b_v, b_t):
    X = np.asarray(X, np.float32)
    T = np.asarray(T, np.float32)
    M = np.asarray(M, np.float32)
    DT = np.asarray(DT, np.float32)
    w_t = np.asarray(w_t, np.float32)
    w_v = np.asarray(w_v, np.float32)
    b_t = np.asarray(b_t, np.float32)
    b_v = np.asarray(b_v, np.float32)
    alpha = np.asarray(alpha, np.float32).reshape(K)

    # weight pack: [K, L, 6, D] with f-order (wt0, wt1, wt3, wt2, 4bt, wv)
    W = np.empty((K, L, NF + 2, D), np.float32)
    W[:, :, 0] = w_t[:, :, :, 0]
    W[:, :, 1] = w_t[:, :, :, 1]
    W[:, :, 2] = w_t[:, :, :, 3]
    W[:, :, 3] = w_t[:, :, :, 2]
    W[:, :, 4] = 4.0 * b_t[:, :, :, 0]
    W[:, :, 5] = w_v
    # -> [K, LP, 6, LJ, D] with l = j*LP + p
    W = W.reshape(K, LJ, LP, NF + 2, D).transpose(0, 2, 3, 1, 4)
    W = np.ascontiguousarray(W).astype(NPBF)

    S = np.tile(
        np.concatenate(
            [-np.maximum(alpha.reshape(1, K), 0.0), -REF_TIME.reshape(1, K)], axis=1
        ),
        (128, 1),
    ).astype(np.float32)
    BV = (float(L) * b_v[:, 0, :]).astype(np.float32)
    ESEL = np.zeros((128, K * K + 128), np.float32)
    for k in range(K):
        ESEL[:, k * K + k] = 1.0
    ESEL[:, K * K :] = np.eye(128, dtype=np.float32)
    ESEL = ESEL.astype(NPBF)

    in_maps = []
    for c in range(NCORES):
        b0 = c * BLOC
        in_maps.append(
            {
                "T": np.ascontiguousarray(T[b0 : b0 + BLOC]),
                "X": np.ascontiguousarray(X[b0 : b0 + BLOC]).astype(NPBF),
                "DTm": np.ascontiguousarray(DT[b0 : b0 + BLOC]).astype(NPBF),
                "Mm": np.ascontiguousarray(M[b0 : b0 + BLOC]).astype(NPBF),
                "W": W,
                "S": S,
                "BV": BV,
                "ESEL": ESEL,
            }
        )
    return in_maps, tuple(bool(a > 0) for a in alpha)


def kernel(X, T, M, DT, alpha, w_v, w_t, b_v, b_t):
    in_maps, nonzero = make_in_maps(X, T, M, DT, alpha, w_v, w_t, b_v, b_t)
    nc = _get_nc(nonzero)
    res = run_bass_kernel_spmd(nc, in_maps, core_ids=list(range(NCORES)))
    out = np.concatenate([res.results[c]["out"] for c in range(NCORES)], axis=0)
    return out.astype(np.float32)
